# revision 80
# baseline (speedup 1.0000x reference)
"""Causal single-head attention (S=8192, dk=64) on 8 TRN2 NeuronCores.

Sharding: zigzag sequence-parallel over query rows. The 8192 rows form 16
blocks of 512; core b owns row-blocks {A=b, B=15-b} so every core does
exactly 17 block-sized (512 rows x 512 keys) units of causal work ->
perfect load balance, no collectives.

SPMD constraint (all cores share one instruction graph) is satisfied by
host-side packing: slots are ordered so slot roles are uniform across
cores:
  slot 0      = diagonal unit of row B (B >= 8, so row B always has >= 9
                units)
  slots 1..8  = (B, c) for c = 0..7            -- always row B
  slots 9..15 = remaining units: (B, c) c=8..B-1 then (A, c) c=0..A-1
                (core-dependent mix, 7 total)
  slot 16     = diagonal unit of row A

Because slots 0..8 are row-B on EVERY core, their AV partials accumulate
directly in one PSUM bank (avB) across the whole kernel -> one copy + one
DMA instead of nine.  Slots 9..16 get a per-slot PSUM tile + copy + DMA;
the host sums those (row-aware) and divides by the denominator row.

Device pipeline per group (group g = slots (g, 9+g), group 8 = slot 8):
  QK^T: per key-subtile: two matmuls [K=64, M=128, N=512] on disjoint PE
        row-groups (tile_position (0,0)/(64,0)) -> sT [128, 1024] f32 in
        PSUM.
  exp:  exp(s/64) -> bf16 SBUF, split between ACT (exact, fused scale)
        and a custom DVE op ((cubic)^2 approx), balanced by the measured
        cost model (ACT ~(172+FD)/1.2 ns, DVE ~(120+FD)/0.96 ns; both
        PSUM-source fp32 = 1x mode).
  mask: diag slots (0 and 16) only: gpsimd affine_select zeroes key>row.
  AV:   per slot: 4 matmuls lhsT=v_aug[128 keys, 65] rhs=exp tile;
        lower slot accumulates into the held avB bank, upper slot into a
        rotating avM bank.  Row 64 of v_aug is ones -> denominator.
  out:  avM -> SBUF copy (ACT or DVE) + DMA per group; avB once at end.

Host combines: per row-block, sum partials (device already summed the 9
row-B slots), divide by denominator row.
"""

import numpy as np
import ml_dtypes

S = 8192
DK = 64
BLK = 512  # row/key block
NB = S // BLK  # 16
N_CORES = 8
NSLOT = 17
G0 = 9  # slots 0..8 -> PE rows 0:64, slots 9..16 -> PE rows 64:128
NGRP = 9  # groups 0..7 = (g, 9+g); group 8 = slot 8 alone
KSUB = 128  # key subtile (psum partition dim)
NKT = BLK // KSUB  # 4
VW = NKT * 65  # 260

# diag groups (0 and 7) have longer exp->mask->AV chains: keep them off
# the cold start and the drain.  Group 8 (single slot, half-sized input)
# is special-cased: its QK+exp run FIRST (the 128KB transfer lands
# earliest, prefilling the pipeline) while its AV matmuls run LAST (so
# the drain chain is just 4 accumulating matmuls + copy + DMA, with no
# QK/exp left in it).
PAIR_ORDER = [1, 2, 3, 0, 4, 5, 7, 6]

_BF16 = ml_dtypes.bfloat16
_CACHE = {}

# cubic-in-t fit of exp(t/128) (chebyshev nodes, |t|<=56); the DVE op
# squares it to get exp(t/64). Max rel err ~5.5e-4 for |t|<=56.
_EXPC = (8.02364796e-08, 3.10070749e-05, 7.81220049e-03, 9.99807965e-01)


def _register_exp_dve_op():
    """Register a custom DVE op: out = (((x*c3 + c2)*x + c1)*x + c0)^2.

    One DVE instruction evaluates exp(x/64) to ~5e-4 rel err, letting the
    Vector engine share softmax-exp work with the Scalar engine.
    """
    import numpy as np
    from concourse import dve_ops
    from concourse.dve_spec import (
        Spec, Src0, C0, C1, C2, C3, _spill_c3_to_src1, lower, _has_src1, sq,
    )
    from concourse.dve_uop import DveOpSpec

    name = "EXP_SQ_ANT"
    if name in dve_ops._SUB_OPCODE_FOR_NAME:
        return next(o for o in dve_ops.OPS if o.name == name)

    body = _spill_c3_to_src1(
        sq(((Src0 * C0 + C1) * Src0 + C2) * Src0 + C3))

    def ref(in0, in1, s0, s1, imm2):
        x = in0.astype(np.float32)
        p = ((x * s0 + s1) * x + imm2) * x + in1
        return (p * p).astype(np.float32)

    spec = Spec(body=body, reference=ref)
    row = dve_ops._CUSTOM_DVE_ROW_BASE + len(dve_ops.OPS)
    assert row < 0x20
    shas = {}
    for ver in ("v3",):
        s = DveOpSpec(name=name, opcode=row, uops=lower(spec, ver=ver),
                      rd1_en=_has_src1(spec))
        shas[ver] = s.sha(ver)
    op = dve_ops.DveOp(name, spec, subdim=False, uops_sha=shas)
    dve_ops.OPS.append(op)
    dve_ops._SUB_OPCODE_FOR_NAME[name] = row
    dve_ops.CUSTOM_DVE_SPECS[name] = spec
    return op


def _core_slots(b):
    """Slot table for core b: list of (rowblock, keyblock, is_diag)."""
    A, B = b, 15 - b
    slots = [(B, B, True)]
    slots += [(B, c, False) for c in range(8)]
    slots += [(B, c, False) for c in range(8, B)]
    slots += [(A, c, False) for c in range(A)]
    slots.append((A, A, True))
    assert len(slots) == NSLOT
    return slots


def _build_graph():
    import concourse.mybir as mybir
    import concourse.tile as tile
    from concourse import bacc

    f32 = mybir.dt.float32
    bf16 = mybir.dt.bfloat16

    exp_op = _register_exp_dve_op()
    d3, d2, d1, d0 = _EXPC

    nc = bacc.Bacc("TRN2", target_bir_lowering=False)
    # qk: per group, q^T strip then k^T strip (each [128, 512], lower slot
    # on partitions 0:64, upper slot on 64:128)
    qkp = nc.declare_dram_parameter("qkp", [NGRP, 128, 2 * BLK], bf16,
                                    isOutput=False)
    vp = nc.declare_dram_parameter("vp", [NGRP, 128, 2 * VW], bf16,
                                   isOutput=False)
    # op[0..7] = per-slot partial of slot 9+g; op[8] = summed row-B (avB)
    op = nc.declare_dram_parameter("op", [NGRP, 65, BLK], f32,
                                   isOutput=True)

    # ACT/DVE load balancer (measured cost model, ns)
    eng_t = {"A": 0.0, "D": 0.0}

    with tile.TileContext(nc) as tc:
        with (
            tc.tile_pool(name="data", bufs=1) as data,
            tc.tile_pool(name="stp", bufs=3, space="PSUM") as stp,
            tc.tile_pool(name="avbp", bufs=1, space="PSUM") as avbp,
            tc.tile_pool(name="avmp", bufs=1, space="PSUM") as avmp,
            tc.tile_pool(name="sxp", bufs=10) as sxp,
            tc.tile_pool(name="outp", bufs=3) as outp,
        ):
            d0col = data.tile([128, 1], f32, tag="d0col", name="d0col")
            nc.vector.memset(d0col, d0)
            qkcol = {}
            vcol = {}
            # DMA issue order matches consumption order: descriptor
            # generation (~600ns/call) is FIFO on the SP HWDGE ring
            for g in [8] + PAIR_ORDER:
                t = data.tile([128, 2 * BLK], bf16, tag=f"qk{g}",
                              name=f"qk{g}")
                vt = data.tile([128, 2 * VW], bf16, tag=f"v{g}",
                               name=f"vc{g}")
                qkcol[g] = t
                vcol[g] = vt
                if g == 8:
                    nc.sync.dma_start(out=t[0:64, :], in_=qkp[8][0:64, :])
                    nc.sync.dma_start(out=vt[:, 0:VW], in_=vp[8][:, 0:VW])
                else:
                    nc.sync.dma_start(out=t, in_=qkp[g])
                    nc.sync.dma_start(out=vt, in_=vp[g])

            avb = avbp.tile([65, BLK], f32, tag="avb", name="avb")
            avb_n = [0]  # emitted avB matmuls (total 9 slots * 4 kt = 36)

            def emit_exp(out_ap, in_ap, fd):
                ca = (172.0 + fd) / 1.2
                cd = (120.0 + fd) / 0.96
                if eng_t["A"] + ca <= eng_t["D"] + cd:
                    eng_t["A"] += ca
                    nc.scalar.activation(
                        out_ap, in_ap,
                        mybir.ActivationFunctionType.Exp,
                        scale=1.0 / DK,
                    )
                else:
                    eng_t["D"] += cd
                    nc.vector._custom_dve(
                        exp_op, out=out_ap, in0=in_ap, in1=d0col,
                        s0=d3, s1=d2, imm2=d1,
                    )

            def emit_copy(out_ap, in_ap, fd):
                ca = (172.0 + fd) / 1.2
                cd = (120.0 + fd) / 0.96
                if eng_t["A"] + ca <= eng_t["D"] + cd:
                    eng_t["A"] += ca
                    nc.scalar.copy(out_ap, in_ap)
                else:
                    eng_t["D"] += cd
                    nc.vector.tensor_copy(out_ap, in_ap)

            def emit_qk_exp(g):
                slots = [g] + ([9 + g] if g < 8 else [])
                sxs = []
                for kt in range(NKT):
                    roff = KSUB * kt
                    # one shared 2-bank score tile per (group, kt): both
                    # row-half matmuls share its WAR dependency, which
                    # keeps them adjacent in the PE queue -> they overlap
                    # in the array (disjoint row groups)
                    st = stp.tile([128, 2 * BLK], f32, tag="st",
                                  name=f"st{g}k{kt}")
                    for s in slots:
                        p0 = 0 if s < G0 else 64
                        off = 0 if s == g else BLK
                        nc.tensor.matmul(
                            st[:, off:off + BLK],
                            qkcol[g][p0:p0 + 64,
                                     BLK + kt * KSUB:BLK + (kt + 1) * KSUB],
                            qkcol[g][p0:p0 + 64, 0:BLK],
                            start=True,
                            stop=True,
                            tile_position=(p0, 0),
                        )
                    if g == 8:
                        # held in the bufs=1 pool: consumed only by the
                        # deferred AV at the very end of the kernel
                        sx = data.tile([128, BLK], bf16, tag=f"sx8k{kt}",
                                       name=f"sx8k{kt}")
                    else:
                        sx = sxp.tile([128, 2 * BLK], bf16, tag="sx",
                                      name=f"sx{g}k{kt}")
                    if g == 0 and kt > 0:
                        spans = [(roff, 2 * BLK)]
                    elif g == 7 and kt > 0:
                        spans = [(0, BLK), (BLK + roff, 2 * BLK)]
                    elif g == 8:
                        spans = [(0, BLK)]
                    else:
                        spans = [(0, 2 * BLK)]
                    for lo, hi in spans:
                        emit_exp(sx[:, lo:hi], st[:, lo:hi], hi - lo)
                    if g == 0:  # diag slot 0 (lower half): zero key > row
                        nc.gpsimd.affine_select(
                            out=sx[:, roff:BLK],
                            in_=sx[:, roff:BLK],
                            pattern=[[1, BLK - roff]],
                            compare_op=mybir.AluOpType.is_ge,
                            fill=0.0,
                            base=0,
                            channel_multiplier=-1,
                        )
                    if g == 7:  # diag slot 16 (upper half)
                        nc.gpsimd.affine_select(
                            out=sx[:, BLK + roff:2 * BLK],
                            in_=sx[:, BLK + roff:2 * BLK],
                            pattern=[[1, BLK - roff]],
                            compare_op=mybir.AluOpType.is_ge,
                            fill=0.0,
                            base=0,
                            channel_multiplier=-1,
                        )
                    sxs.append(sx)
                return sxs

            def emit_av_out(g, sxs):
                # lower slot g -> accumulate into the held avB bank
                for kt in range(NKT):
                    roff = KSUB * kt if g == 0 else 0
                    avb_n[0] += 1
                    nc.tensor.matmul(
                        avb[:, roff:BLK],
                        vcol[g][:, kt * 65:(kt + 1) * 65],
                        sxs[kt][:, roff:BLK],
                        start=(avb_n[0] == 1),
                        stop=(avb_n[0] == 36),
                    )
                if g == 8:
                    return
                # upper slot 9+g -> rotating avM bank, copy + DMA
                avm = avmp.tile([65, BLK], f32, tag="avm", name=f"avm{g}")
                for kt in range(NKT):
                    roff = KSUB * kt if g == 7 else 0
                    nc.tensor.matmul(
                        avm[:, roff:BLK],
                        vcol[g][:, VW + kt * 65:VW + (kt + 1) * 65],
                        sxs[kt][:, BLK + roff:2 * BLK],
                        start=(kt == 0), stop=(kt == NKT - 1),
                    )
                ot = outp.tile([65, BLK], f32, tag="ot", name=f"ot{g}")
                emit_copy(ot, avm, BLK)
                nc.sync.dma_start(out=op[g], in_=ot)

            # group 8's QK+exp run first (earliest data, prefills the
            # pipeline); its AV is deferred to the very end so the drain
            # holds no QK/exp
            sxs8 = emit_qk_exp(8)
            # software-pipelined emission: group p's AV stage is emitted
            # after group p+1's QK+exp stage so AVs fill dependency stalls
            prev = None
            for g in PAIR_ORDER:
                sxs = emit_qk_exp(g)
                if prev is not None:
                    emit_av_out(*prev)
                prev = (g, sxs)
            emit_av_out(*prev)
            emit_av_out(8, sxs8)
            assert avb_n[0] == 36
            otb = outp.tile([65, BLK], f32, tag="ot", name="otb")
            emit_copy(otb, avb, BLK)
            nc.sync.dma_start(out=op[8], in_=otb)

    nc.finalize()
    return nc


def _pack_core(q_bf, k_bf, v_bf, b):
    """Build the packed operand arrays for core b."""
    qkp = np.zeros((NGRP, 128, 2 * BLK), dtype=_BF16)
    vp = np.zeros((NGRP, 128, 2 * VW), dtype=_BF16)
    slots = _core_slots(b)
    for s, (rb, cb, _diag) in enumerate(slots):
        g = s if s < G0 else s - G0
        p0 = 0 if s < G0 else 64
        voff = 0 if s < G0 else VW
        qkp[g, p0:p0 + 64, 0:BLK] = q_bf[rb * BLK:(rb + 1) * BLK].T
        qkp[g, p0:p0 + 64, BLK:2 * BLK] = k_bf[cb * BLK:(cb + 1) * BLK].T
        for kt in range(NKT):
            c0 = voff + kt * 65
            vp[g, :, c0:c0 + 64] = (
                v_bf[cb * BLK + kt * KSUB: cb * BLK + (kt + 1) * KSUB])
            vp[g, :, c0 + 64] = np.asarray(1.0, dtype=_BF16)
    return {"qkp": qkp, "vp": vp}


def _combine(partials):
    """partials: list of 8 arrays [9, 65, 512] f32 -> full [8192, 64]."""
    out = np.empty((S, DK), dtype=np.float32)
    for b in range(N_CORES):
        slots = _core_slots(b)
        A, B = b, 15 - b
        totB = partials[b][8].astype(np.float32).copy()
        totA = np.zeros((65, BLK), dtype=np.float32)
        for g in range(8):
            rb = slots[9 + g][0]
            if rb == B:
                totB += partials[b][g]
            else:
                totA += partials[b][g]
        out[B * BLK:(B + 1) * BLK] = (totB[:DK] / totB[DK]).T
        out[A * BLK:(A + 1) * BLK] = (totA[:DK] / totA[DK]).T
    return out


def kernel(q, k, v):
    from concourse.bass_utils import run_bass_kernel_spmd

    q = np.asarray(q, dtype=np.float32)
    k = np.asarray(k, dtype=np.float32)
    v = np.asarray(v, dtype=np.float32)

    if "nc" not in _CACHE:
        _CACHE["nc"] = _build_graph()
    nc = _CACHE["nc"]

    q_bf = q.astype(_BF16)
    k_bf = k.astype(_BF16)
    v_bf = v.astype(_BF16)
    in_maps = [_pack_core(q_bf, k_bf, v_bf, b) for b in range(N_CORES)]

    res = run_bass_kernel_spmd(nc, in_maps, core_ids=list(range(N_CORES)))
    partials = [np.asarray(res.results[b]["op"], dtype=np.float32)
                for b in range(N_CORES)]
    return _combine(partials)
